# revision 58
# baseline (speedup 1.0000x reference)
"""Trainium2 Bass kernel for nn_Attention_49452253446746.

Full fused attention: qkv projection + interleaved RoPE (with pe_scale) +
masked softmax + attention output, for x(4,2048,1024), 16 heads of d=64.

Sharding: 8 cores = 4 batches x 2 head-groups (8 heads each). Each core
computes out[b, :, g*512:(g+1)*512].

Key layout choices (per core):
- All matmuls in bf16 (fp32 is 4 cycles/row on the PE); ~0.5% rel err.
- Projection computes q/k directly in (d, t) "transposed" layout via
  lhsT=W^T tiles, rhs=x^T tiles; v in natural (t, d) layout.
- RoPE: q' = (q+bq)*A + rot(q+bq)*B'. The pair swap inside rot() is a DVE
  stream_shuffle of the bias-added SBUF tile; the interleave signs are
  folded into the host-built B' tables (A/B' also fold cos/sin, pe_scale
  and the attention scale), so RoPE costs no extra matmuls.
- scores^T[k,q] = k'^T q' (contraction d=64). The two heads of a pair are
  row-tiled at array rows 0-63/64-127 (tile_position) and write the two
  banks of one (128,2,512) psum tile, so they run concurrently on HW and
  a single 1024-wide exp on the scalar engine covers both.
- Softmax denominators ride along as a 65th all-ones column appended to v
  in the p@v matmul; the additive mask is folded as exp(mask) (host) and
  multiplied into exp(scores) on the DVE in bf16 2x mode.
- The kernel outputs the raw (65,512) p@v accumulators; the host divides
  by the sums row and transposes (d,q)->(q,d) while assembling the full
  output, keeping the device free of transpose/divide passes.
- Emission interleaves the next pair's projection and the v projection
  into ACT-bound attention iterations; DMA issue order follows first use.
"""

import sys

for _p in ("/opt/trn_rl_repo",):
    if _p not in sys.path:
        sys.path.insert(0, _p)

import numpy as np
import ml_dtypes

import concourse.bacc as bacc
import concourse.mybir as mybir
from concourse.tile import TileContext
from concourse.masks import make_identity
from concourse.bass_utils import run_bass_kernel_spmd

BF16 = mybir.dt.bfloat16
F32 = mybir.dt.float32
AF = mybir.ActivationFunctionType
ALU = mybir.AluOpType

B, T_FULL, C, NH, D = 4, 2048, 1024, 16, 64
N_CORES = 8
HPC = 8          # heads per core
PAIRS = 4        # head pairs per core
CO = C // 128    # 8 contraction tiles for the projection
JW = 1536        # w columns: q(512) k(512) v(512)
SCALE = 1.0 / float(np.sqrt(2.0 * D))

# stream_shuffle mask: swap adjacent partition pairs within each 32-group
SWAP_MASK = [i ^ 1 for i in range(32)]


def _build(T=T_FULL):
    TC_ = T // 512   # 512-wide q/t chunks
    TT = T // 128    # 128-wide t tiles
    KT = T // 128    # 128-wide k tiles

    nc = bacc.Bacc("TRN2", target_bir_lowering=False, debug=False,
                   num_devices=N_CORES)
    xT = nc.dram_tensor("xT", (C, T), BF16, kind="ExternalInput")
    wT = nc.dram_tensor("wT", (C, JW), BF16, kind="ExternalInput")
    ab = nc.dram_tensor("ab", (128, 4, T), BF16, kind="ExternalInput")
    bias = nc.dram_tensor("bias", (128, 16), F32, kind="ExternalInput")
    bv = nc.dram_tensor("bv", (1, 512), BF16, kind="ExternalInput")
    em = nc.dram_tensor("em", (T, T), BF16, kind="ExternalInput")
    out = nc.dram_tensor("out", (HPC, T // 512, 65, 512), F32,
                         kind="ExternalOutput")

    with TileContext(nc) as tc:
        with (
            tc.tile_pool(name="const", bufs=1) as constp,
            tc.tile_pool(name="work", bufs=2) as workp,
            tc.tile_pool(name="eb", bufs=6) as ebp,
            tc.tile_pool(name="pp", bufs=2, space="PSUM") as proj_ps,
            tc.tile_pool(name="sp", bufs=2, space="PSUM") as score_ps,
            tc.tile_pool(name="vp", bufs=1, space="PSUM") as pv_ps,
        ):
            x_sb = constp.tile([128, CO, T], BF16)
            xT_r = xT.ap().rearrange("(co p) t -> p co t", p=128)
            w_sb = constp.tile([128, CO, JW], BF16)
            wT_r = wT.ap().rearrange("(co p) j -> p co j", p=128)
            bias_sb = constp.tile([128, 16], F32)
            nc.sync.dma_start(bias_sb, bias.ap())
            bv_sb = constp.tile([1, 512], BF16)
            nc.sync.dma_start(bv_sb, bv.ap())
            # minimal working set first: x + pair-0 q/k weight columns.
            # One descriptor per logical chunk — the HWDGE descriptor pipe
            # serializes at ~0.6us per dma_start, so fewer, bigger DMAs win.
            nc.sync.dma_start(w_sb[:, :, 0:128], wT_r[:, :, 0:128])
            nc.sync.dma_start(w_sb[:, :, 512:640], wT_r[:, :, 512:640])
            for co in range(CO):
                nc.sync.dma_start(x_sb[:, co], xT_r[:, co])
            ab_sb = constp.tile([128, 4, T], BF16)
            nc.sync.dma_start(ab_sb[:, 0:2], ab.ap()[:, 0:2])
            nc.sync.dma_start(ab_sb[:, 2:4], ab.ap()[:, 2:4])
            nc.sync.dma_start(w_sb[:, :, 1024:1536], wT_r[:, :, 1024:1536])
            ones1 = constp.tile([1, 128], BF16)
            nc.vector.memset(ones1, 1.0)

            # v with a 65th all-ones column per head (softmax denominators)
            v_sb = constp.tile([128, KT, HPC * 65], BF16)
            v_sb_h = v_sb.rearrange("p k (h y) -> p k h y", y=65)
            nc.vector.memset(v_sb_h[:, :, :, 64], 1.0)

            # q'/k' in (d, t) layout; j-tiles: q pairs 0-3, k pairs 4-7
            qk_sb = constp.tile([128, 8, T], BF16)

            # ---- projection + rope (emitted per pair, per t-chunk) ----
            def proj_tc(p_, tcx):
                    tsl = slice(tcx * 512, (tcx + 1) * 512)
                    for joff, aoff, boff, dst in (
                        (p_, 0, 0, p_),          # q
                        (4 + p_, 2, 8, 4 + p_),  # k
                    ):
                        ps_m = proj_ps.tile([128, 512], F32, tag="pp")
                        for co in range(CO):
                            nc.tensor.matmul(
                                ps_m,
                                w_sb[:, co, joff * 128:(joff + 1) * 128],
                                x_sb[:, co, tsl],
                                start=(co == 0), stop=(co == CO - 1))
                        # qsb = q + bias (psum -> sbuf); then the shuffle of
                        # qsb already carries swap(bias), so the B' multiply
                        # needs no extra bias term.
                        qsb = workp.tile([128, 512], F32, tag="qsb")
                        nc.vector.tensor_scalar_add(
                            qsb, ps_m, bias_sb[:, boff + p_:boff + p_ + 1])
                        shuf = workp.tile([128, 512], F32, tag="shuf")
                        nc.vector.stream_shuffle(shuf, qsb, SWAP_MASK)
                        s1 = workp.tile([128, 512], F32, tag="s1")
                        nc.vector.tensor_mul(s1, qsb, ab_sb[:, aoff, tsl])
                        s2 = workp.tile([128, 512], F32, tag="s2")
                        nc.vector.tensor_mul(s2, shuf, ab_sb[:, aoff + 1, tsl])
                        nc.gpsimd.tensor_add(qk_sb[:, dst, tsl], s1, s2)

            # ---- v projection (per t-tile; interleavable) ----
            def proj_pair(p_):
                for tcx in range(TC_):
                    proj_tc(p_, tcx)

            def v_proj(tt):
                psv = proj_ps.tile([128, 512], F32, tag="pp", name="psv")
                for co in range(CO):
                    nc.tensor.matmul(
                        psv,
                        x_sb[:, co, tt * 128:(tt + 1) * 128],
                        w_sb[:, co, 1024:1536],
                        start=(co == 0), stop=False)
                # bias via K=1 matmul: psv[t, j] += 1 * bv[j]
                nc.tensor.matmul(psv, ones1, bv_sb, start=False, stop=True)
                nc.vector.tensor_copy(
                    v_sb_h[:, tt, :, :64],
                    psv.rearrange("p (h d) -> p h d", d=64))

            # ---- attention (pair-outer; proj/v of later pairs overlap) ----
            proj_pair(0)
            v_proj(0)
            em_r = em.ap()
            for p_ in range(PAIRS):
                for qcx in range(TC_):
                    qsl = slice(qcx * 512, (qcx + 1) * 512)
                    em_t = workp.tile([128, KT, 512], BF16, tag="em")
                    # chunk only the first tile (arrival granularity);
                    # single-descriptor DMAs otherwise
                    ech = KT if (p_ or qcx) else max(1, KT // 4)
                    for ec in range(KT // ech):
                        nc.sync.dma_start(
                            em_t[:, ec * ech:(ec + 1) * ech],
                            em_r[ec * ech * 128:(ec + 1) * ech * 128,
                                 qsl].rearrange("(kt p) q -> p kt q", p=128))
                    if p_ == 0:
                        # stage later pairs' q/k weight columns, spread over qc
                        for pp in range(1, PAIRS):
                            if min(pp - 1, TC_ - 1) != qcx:
                                continue
                            qa, qb = pp * 128, (pp + 1) * 128
                            nc.sync.dma_start(w_sb[:, :, qa:qb],
                                              wT_r[:, :, qa:qb])
                            nc.sync.dma_start(
                                w_sb[:, :, 512 + qa:512 + qb],
                                wT_r[:, :, 512 + qa:512 + qb])
                    pvs = [pv_ps.tile([65, 512], F32, tag=f"pv{hh}",
                                      name=f"pv{hh}") for hh in range(2)]
                    for kt in range(KT):
                        sc = score_ps.tile([128, 2, 512], F32, tag="sc")
                        for hh in range(2):
                            pb = hh * 64
                            # the second (rows 64-127) matmul of the pair is
                            # high-priority so no other matmul lands between
                            # the two: adjacency preserves their row-group
                            # concurrency on hardware.
                            import contextlib
                            hp = tc.high_priority() if hh else contextlib.nullcontext()
                            with hp:
                                nc.tensor.matmul(
                                    sc[:, hh],
                                    qk_sb[pb:pb + 64, 4 + p_,
                                          kt * 128:(kt + 1) * 128],
                                    qk_sb[pb:pb + 64, p_, qsl],
                                    start=True, stop=True,
                                    tile_position=(pb, 0))
                        e_t = ebp.tile([128, 2, 512], BF16, tag="e")
                        nc.scalar.activation(e_t, sc, AF.Exp)
                        ep_t = ebp.tile([128, 2, 512], BF16, tag="ep")
                        emb = em_t[:, kt:kt + 1, :].to_broadcast((128, 2, 512))
                        nc.vector.tensor_mul(ep_t, e_t, emb)
                        if p_ == 0 and qcx == 0 and 1 <= kt:
                            v_proj(kt)  # stream the rest of v in
                        for hh in range(2):
                            h = 2 * p_ + hh
                            nc.tensor.matmul(
                                pvs[hh],
                                v_sb_h[:, kt, h, :],
                                ep_t[:, hh],
                                start=(kt == 0),
                                stop=(kt == KT - 1))
                    for hh in range(2):
                        o_sb = workp.tile([65, 512], F32, tag="osb")
                        nc.vector.tensor_copy(o_sb, pvs[hh])
                        nc.sync.dma_start(out.ap()[2 * p_ + hh, qcx], o_sb)
                    if p_ + 1 < PAIRS:
                        for tcx2 in range(TC_):
                            if min(tcx2, TC_ - 1) == qcx or (
                                    tcx2 >= TC_ - 1 and qcx == TC_ - 1):
                                if tcx2 == qcx or qcx == TC_ - 1 and tcx2 >= qcx:
                                    proj_tc(p_ + 1, tcx2)
    nc.compile()
    return nc


def _host_prep(inputs, T=T_FULL):
    bf = ml_dtypes.bfloat16
    x = np.asarray(inputs["x"], np.float32)
    pe_cos = np.asarray(inputs["pe_cos"], np.float32)[0, 0]      # (T, D)
    pe_sin = np.asarray(inputs["pe_sin"], np.float32)[0, 0]
    pe_scale = np.asarray(inputs["pe_scale"], np.float32)[0, 0]
    mask = np.asarray(inputs["mask"], np.float32)[0]             # (B, T, T)
    w = np.asarray(inputs["w_qkv"], np.float32)                  # (3C, C)
    b = np.asarray(inputs["b_qkv"], np.float32)

    cosT, sinT, scT = pe_cos.T, pe_sin.T, pe_scale.T             # (D, T)
    # sign pattern folded into the B tables: rot(u)[d] = sgn[d]*u[d^1]
    sgn = np.tile(np.array([-1.0, 1.0], np.float32), D // 2)[:, None]
    ab_host = np.stack([
        np.tile(cosT * scT * SCALE, (2, 1)),
        np.tile(sinT * scT * SCALE * sgn, (2, 1)),
        np.tile(cosT / scT, (2, 1)),
        np.tile(sinT / scT * sgn, (2, 1)),
    ], axis=1).astype(bf)                                        # (128, 4, T)

    def swap_pairs(v):
        return np.ascontiguousarray(v.reshape(-1, 2)[:, ::-1]).reshape(v.shape)

    in_maps = []
    for c in range(N_CORES):
        bidx, g = divmod(c, 2)
        gs = slice(g * 512, (g + 1) * 512)
        wq, wk, wv = w[:C][gs], w[C:2 * C][gs], w[2 * C:][gs]
        bq, bk, bv_ = b[:C][gs], b[C:2 * C][gs], b[2 * C:][gs]
        wT_host = np.ascontiguousarray(
            np.concatenate([wq, wk, wv], 0).T).astype(bf)
        # bias cols: 0-3 q, 4-7 swap(bq), 8-11 k, 12-15 swap(bk)
        bias16 = np.zeros((128, 16), np.float32)
        bqs, bks = swap_pairs(bq), swap_pairs(bk)
        for p_ in range(PAIRS):
            ps = slice(p_ * 128, (p_ + 1) * 128)
            bias16[:, p_] = bq[ps]
            bias16[:, 4 + p_] = bqs[ps]
            bias16[:, 8 + p_] = bk[ps]
            bias16[:, 12 + p_] = bks[ps]
        in_maps.append({
            "xT": np.ascontiguousarray(x[bidx].T).astype(bf),
            "wT": wT_host,
            "ab": ab_host,
            "bias": bias16,
            "bv": bv_.astype(bf)[None, :],
            "em": np.exp(mask[bidx].T).astype(bf),
        })
    return in_maps


def _run_spmd(in_maps):
    nc = _build()
    res = run_bass_kernel_spmd(nc, in_maps, core_ids=list(range(N_CORES)))
    return np.stack([res.results[c]["out"] for c in range(N_CORES)])


def _assemble(outs):
    full = np.empty((B, T_FULL, C), np.float32)
    for c in range(N_CORES):
        bidx, g = divmod(c, 2)
        a = outs[c]                                  # (HPC, TC, 65, 512)
        o = a[:, :, :64, :] / a[:, :, 64:65, :]      # divide by softmax sums
        # [h, qc, d, q] -> [qc*512+q, h*64+d]
        full[bidx, :, g * 512:(g + 1) * 512] = (
            o.transpose(1, 3, 0, 2).reshape(T_FULL, 512))
    return full


def _run_spmd_main():
    """Subprocess entry: read in_maps npz from argv[1], write outs to argv[2]."""
    z = np.load(sys.argv[1])
    in_maps = []
    for c in range(N_CORES):
        m = {}
        for k in z.files:
            if not k.startswith(f"{c}_"):
                continue
            name = k[k.index("_") + 1:]
            v = z[k]
            if v.dtype.kind == "V":  # bf16 round-trips as void16
                v = v.view(ml_dtypes.bfloat16)
            m[name] = v
        in_maps.append(m)
    outs = _run_spmd(in_maps)
    np.save(sys.argv[2], outs)


def kernel(**inputs):
    in_maps = _host_prep(inputs)
    err = None
    for attempt in range(2):
        try:
            return _assemble(_run_spmd(in_maps))
        except Exception as e:          # noqa: BLE001 - device flakiness
            err = e
    # The axon device occasionally wedges for the rest of the process
    # (NRT_EXEC_UNIT_UNRECOVERABLE); a fresh process recovers it.
    import os
    import subprocess
    import tempfile
    here = os.path.dirname(os.path.abspath(__file__))
    for attempt in range(2):
        with tempfile.TemporaryDirectory() as td:
            inp, outp = os.path.join(td, "in.npz"), os.path.join(td, "out.npy")
            np.savez(inp, **{f"{c}_{k}": v for c, m in enumerate(in_maps)
                             for k, v in m.items()})
            r = subprocess.run(
                [sys.executable, "-c",
                 "import sys; sys.path.insert(0, sys.argv[3]); "
                 "import kernel; kernel._run_spmd_main()",
                 inp, outp, here],
                cwd=here, capture_output=True, text=True, timeout=1800)
            if r.returncode == 0 and os.path.exists(outp):
                return _assemble(np.load(outp))
            err = RuntimeError(
                f"subprocess kernel attempt failed: {r.stderr[-2000:]}")
    raise err


# revision 59
# speedup vs baseline: 1.0150x; 1.0150x over previous
"""Trainium2 Bass kernel for nn_Attention_49452253446746.

Full fused attention: qkv projection + interleaved RoPE (with pe_scale) +
masked softmax + attention output, for x(4,2048,1024), 16 heads of d=64.

Sharding: 8 cores = 4 batches x 2 head-groups (8 heads each). Each core
computes out[b, :, g*512:(g+1)*512].

Key layout choices (per core):
- All matmuls in bf16 (fp32 is 4 cycles/row on the PE); ~0.5% rel err.
- Projection computes q/k directly in (d, t) "transposed" layout via
  lhsT=W^T tiles, rhs=x^T tiles; v in natural (t, d) layout.
- RoPE: q' = (q+bq)*A + rot(q+bq)*B'. The pair swap inside rot() is a DVE
  stream_shuffle of the bias-added SBUF tile; the interleave signs are
  folded into the host-built B' tables (A/B' also fold cos/sin, pe_scale
  and the attention scale), so RoPE costs no extra matmuls.
- scores^T[k,q] = k'^T q' (contraction d=64). The two heads of a pair are
  row-tiled at array rows 0-63/64-127 (tile_position) and write the two
  banks of one (128,2,512) psum tile, so they run concurrently on HW and
  a single 1024-wide exp on the scalar engine covers both.
- Softmax denominators ride along as a 65th all-ones column appended to v
  in the p@v matmul; the additive mask is folded as exp(mask) (host) and
  multiplied into exp(scores) on the DVE in bf16 2x mode.
- The kernel outputs the raw (65,512) p@v accumulators; the host divides
  by the sums row and transposes (d,q)->(q,d) while assembling the full
  output, keeping the device free of transpose/divide passes.
- Emission interleaves the next pair's projection and the v projection
  into ACT-bound attention iterations; DMA issue order follows first use.
"""

import sys

for _p in ("/opt/trn_rl_repo",):
    if _p not in sys.path:
        sys.path.insert(0, _p)

import numpy as np
import ml_dtypes

import concourse.bacc as bacc
import concourse.mybir as mybir
from concourse.tile import TileContext
from concourse.masks import make_identity
from concourse.bass_utils import run_bass_kernel_spmd

BF16 = mybir.dt.bfloat16
F32 = mybir.dt.float32
AF = mybir.ActivationFunctionType
ALU = mybir.AluOpType

B, T_FULL, C, NH, D = 4, 2048, 1024, 16, 64
N_CORES = 8
HPC = 8          # heads per core
PAIRS = 4        # head pairs per core
CO = C // 128    # 8 contraction tiles for the projection
JW = 1536        # w columns: q(512) k(512) v(512)
SCALE = 1.0 / float(np.sqrt(2.0 * D))

# stream_shuffle mask: swap adjacent partition pairs within each 32-group
SWAP_MASK = [i ^ 1 for i in range(32)]


def _build(T=T_FULL):
    TC_ = T // 512   # 512-wide q/t chunks
    TT = T // 128    # 128-wide t tiles
    KT = T // 128    # 128-wide k tiles

    nc = bacc.Bacc("TRN2", target_bir_lowering=False, debug=False,
                   num_devices=N_CORES)
    xT = nc.dram_tensor("xT", (C, T), BF16, kind="ExternalInput")
    wT = nc.dram_tensor("wT", (C, JW), BF16, kind="ExternalInput")
    ab = nc.dram_tensor("ab", (128, 4, T), BF16, kind="ExternalInput")
    bias = nc.dram_tensor("bias", (128, 16), F32, kind="ExternalInput")
    bv = nc.dram_tensor("bv", (1, 512), BF16, kind="ExternalInput")
    em = nc.dram_tensor("em", (T, T), BF16, kind="ExternalInput")
    out = nc.dram_tensor("out", (HPC, T // 512, 65, 512), F32,
                         kind="ExternalOutput")

    with TileContext(nc) as tc:
        with (
            tc.tile_pool(name="const", bufs=1) as constp,
            tc.tile_pool(name="work", bufs=2) as workp,
            tc.tile_pool(name="eb", bufs=6) as ebp,
            tc.tile_pool(name="pp", bufs=2, space="PSUM") as proj_ps,
            tc.tile_pool(name="sp", bufs=2, space="PSUM") as score_ps,
            tc.tile_pool(name="vp", bufs=1, space="PSUM") as pv_ps,
        ):
            x_sb = constp.tile([128, CO, T], BF16)
            xT_r = xT.ap().rearrange("(co p) t -> p co t", p=128)
            w_sb = constp.tile([128, CO, JW], BF16)
            wT_r = wT.ap().rearrange("(co p) j -> p co j", p=128)
            bias_sb = constp.tile([128, 16], F32)
            nc.sync.dma_start(bias_sb, bias.ap())
            bv_sb = constp.tile([1, 512], BF16)
            nc.sync.dma_start(bv_sb, bv.ap())
            # minimal working set first: x + pair-0 q/k weight columns.
            # One descriptor per logical chunk — the HWDGE descriptor pipe
            # serializes at ~0.6us per dma_start, so fewer, bigger DMAs win.
            nc.sync.dma_start(w_sb[:, :, 0:128], wT_r[:, :, 0:128])
            nc.sync.dma_start(w_sb[:, :, 512:640], wT_r[:, :, 512:640])
            for co in range(CO):
                nc.sync.dma_start(x_sb[:, co], xT_r[:, co])
            ab_sb = constp.tile([128, 4, T], BF16)
            nc.sync.dma_start(ab_sb[:, 0:2], ab.ap()[:, 0:2])
            nc.sync.dma_start(ab_sb[:, 2:4], ab.ap()[:, 2:4])
            nc.sync.dma_start(w_sb[:, :, 1024:1536], wT_r[:, :, 1024:1536])
            ones1 = constp.tile([1, 128], BF16)
            nc.vector.memset(ones1, 1.0)

            # v with a 65th all-ones column per head (softmax denominators)
            v_sb = constp.tile([128, KT, HPC * 65], BF16)
            v_sb_h = v_sb.rearrange("p k (h y) -> p k h y", y=65)
            nc.vector.memset(v_sb_h[:, :, :, 64], 1.0)

            # q'/k' in (d, t) layout; j-tiles: q pairs 0-3, k pairs 4-7
            qk_sb = constp.tile([128, 8, T], BF16)

            # ---- projection + rope (emitted per pair, per t-chunk) ----
            def proj_tc(p_, tcx):
                    tsl = slice(tcx * 512, (tcx + 1) * 512)
                    for joff, aoff, boff, dst in (
                        (p_, 0, 0, p_),          # q
                        (4 + p_, 2, 8, 4 + p_),  # k
                    ):
                        ps_m = proj_ps.tile([128, 512], F32, tag="pp")
                        for co in range(CO):
                            nc.tensor.matmul(
                                ps_m,
                                w_sb[:, co, joff * 128:(joff + 1) * 128],
                                x_sb[:, co, tsl],
                                start=(co == 0), stop=(co == CO - 1))
                        # qsb = q + bias (psum -> sbuf); then the shuffle of
                        # qsb already carries swap(bias), so the B' multiply
                        # needs no extra bias term.
                        qsb = workp.tile([128, 512], F32, tag="qsb")
                        nc.vector.tensor_scalar_add(
                            qsb, ps_m, bias_sb[:, boff + p_:boff + p_ + 1])
                        shuf = workp.tile([128, 512], F32, tag="shuf")
                        nc.vector.stream_shuffle(shuf, qsb, SWAP_MASK)
                        s1 = workp.tile([128, 512], F32, tag="s1")
                        nc.vector.tensor_mul(s1, qsb, ab_sb[:, aoff, tsl])
                        s2 = workp.tile([128, 512], F32, tag="s2")
                        nc.vector.tensor_mul(s2, shuf, ab_sb[:, aoff + 1, tsl])
                        nc.gpsimd.tensor_add(qk_sb[:, dst, tsl], s1, s2)

            # ---- v projection (per t-tile; interleavable) ----
            def proj_pair(p_):
                for tcx in range(TC_):
                    proj_tc(p_, tcx)

            def v_proj(tt):
                psv = proj_ps.tile([128, 512], F32, tag="pp", name="psv")
                for co in range(CO):
                    nc.tensor.matmul(
                        psv,
                        x_sb[:, co, tt * 128:(tt + 1) * 128],
                        w_sb[:, co, 1024:1536],
                        start=(co == 0), stop=False)
                # bias via K=1 matmul: psv[t, j] += 1 * bv[j]
                nc.tensor.matmul(psv, ones1, bv_sb, start=False, stop=True)
                nc.vector.tensor_copy(
                    v_sb_h[:, tt, :, :64],
                    psv.rearrange("p (h d) -> p h d", d=64))

            # ---- attention (pair-outer; proj/v of later pairs overlap) ----
            proj_pair(0)
            v_proj(0)
            em_r = em.ap()
            for p_ in range(PAIRS):
                for qcx in range(TC_):
                    qsl = slice(qcx * 512, (qcx + 1) * 512)
                    em_t = workp.tile([128, KT, 512], BF16, tag="em")
                    # chunk only the first tile (arrival granularity);
                    # single-descriptor DMAs otherwise
                    ech = KT if (p_ or qcx) else max(1, KT // 4)
                    for ec in range(KT // ech):
                        nc.sync.dma_start(
                            em_t[:, ec * ech:(ec + 1) * ech],
                            em_r[ec * ech * 128:(ec + 1) * ech * 128,
                                 qsl].rearrange("(kt p) q -> p kt q", p=128))
                    if p_ == 0:
                        # stage later pairs' q/k weight columns, spread over qc
                        for pp in range(1, PAIRS):
                            if min(pp - 1, TC_ - 1) != qcx:
                                continue
                            qa, qb = pp * 128, (pp + 1) * 128
                            nc.sync.dma_start(w_sb[:, :, qa:qb],
                                              wT_r[:, :, qa:qb])
                            nc.sync.dma_start(
                                w_sb[:, :, 512 + qa:512 + qb],
                                wT_r[:, :, 512 + qa:512 + qb])
                    pvs = [pv_ps.tile([65, 512], F32, tag=f"pv{hh}",
                                      name=f"pv{hh}") for hh in range(2)]
                    for kt in range(KT):
                        sc = score_ps.tile([128, 2, 512], F32, tag="sc")
                        for hh in range(2):
                            pb = hh * 64
                            # the second (rows 64-127) matmul of the pair is
                            # high-priority so no other matmul lands between
                            # the two: adjacency preserves their row-group
                            # concurrency on hardware.
                            with tc.high_priority():
                                nc.tensor.matmul(
                                    sc[:, hh],
                                    qk_sb[pb:pb + 64, 4 + p_,
                                          kt * 128:(kt + 1) * 128],
                                    qk_sb[pb:pb + 64, p_, qsl],
                                    start=True, stop=True,
                                    tile_position=(pb, 0))
                        e_t = ebp.tile([128, 2, 512], BF16, tag="e")
                        nc.scalar.activation(e_t, sc, AF.Exp)
                        ep_t = ebp.tile([128, 2, 512], BF16, tag="ep")
                        emb = em_t[:, kt:kt + 1, :].to_broadcast((128, 2, 512))
                        nc.vector.tensor_mul(ep_t, e_t, emb)
                        if p_ == 0 and qcx == 0 and 1 <= kt:
                            v_proj(kt)  # stream the rest of v in
                        for hh in range(2):
                            h = 2 * p_ + hh
                            nc.tensor.matmul(
                                pvs[hh],
                                v_sb_h[:, kt, h, :],
                                ep_t[:, hh],
                                start=(kt == 0),
                                stop=(kt == KT - 1))
                    for hh in range(2):
                        o_sb = workp.tile([65, 512], F32, tag="osb")
                        nc.vector.tensor_copy(o_sb, pvs[hh])
                        nc.sync.dma_start(out.ap()[2 * p_ + hh, qcx], o_sb)
                    if p_ + 1 < PAIRS:
                        for tcx2 in range(TC_):
                            if min(tcx2, TC_ - 1) == qcx or (
                                    tcx2 >= TC_ - 1 and qcx == TC_ - 1):
                                if tcx2 == qcx or qcx == TC_ - 1 and tcx2 >= qcx:
                                    proj_tc(p_ + 1, tcx2)
    nc.compile()
    return nc


def _host_prep(inputs, T=T_FULL):
    bf = ml_dtypes.bfloat16
    x = np.asarray(inputs["x"], np.float32)
    pe_cos = np.asarray(inputs["pe_cos"], np.float32)[0, 0]      # (T, D)
    pe_sin = np.asarray(inputs["pe_sin"], np.float32)[0, 0]
    pe_scale = np.asarray(inputs["pe_scale"], np.float32)[0, 0]
    mask = np.asarray(inputs["mask"], np.float32)[0]             # (B, T, T)
    w = np.asarray(inputs["w_qkv"], np.float32)                  # (3C, C)
    b = np.asarray(inputs["b_qkv"], np.float32)

    cosT, sinT, scT = pe_cos.T, pe_sin.T, pe_scale.T             # (D, T)
    # sign pattern folded into the B tables: rot(u)[d] = sgn[d]*u[d^1]
    sgn = np.tile(np.array([-1.0, 1.0], np.float32), D // 2)[:, None]
    ab_host = np.stack([
        np.tile(cosT * scT * SCALE, (2, 1)),
        np.tile(sinT * scT * SCALE * sgn, (2, 1)),
        np.tile(cosT / scT, (2, 1)),
        np.tile(sinT / scT * sgn, (2, 1)),
    ], axis=1).astype(bf)                                        # (128, 4, T)

    def swap_pairs(v):
        return np.ascontiguousarray(v.reshape(-1, 2)[:, ::-1]).reshape(v.shape)

    in_maps = []
    for c in range(N_CORES):
        bidx, g = divmod(c, 2)
        gs = slice(g * 512, (g + 1) * 512)
        wq, wk, wv = w[:C][gs], w[C:2 * C][gs], w[2 * C:][gs]
        bq, bk, bv_ = b[:C][gs], b[C:2 * C][gs], b[2 * C:][gs]
        wT_host = np.ascontiguousarray(
            np.concatenate([wq, wk, wv], 0).T).astype(bf)
        # bias cols: 0-3 q, 4-7 swap(bq), 8-11 k, 12-15 swap(bk)
        bias16 = np.zeros((128, 16), np.float32)
        bqs, bks = swap_pairs(bq), swap_pairs(bk)
        for p_ in range(PAIRS):
            ps = slice(p_ * 128, (p_ + 1) * 128)
            bias16[:, p_] = bq[ps]
            bias16[:, 4 + p_] = bqs[ps]
            bias16[:, 8 + p_] = bk[ps]
            bias16[:, 12 + p_] = bks[ps]
        in_maps.append({
            "xT": np.ascontiguousarray(x[bidx].T).astype(bf),
            "wT": wT_host,
            "ab": ab_host,
            "bias": bias16,
            "bv": bv_.astype(bf)[None, :],
            "em": np.exp(mask[bidx].T).astype(bf),
        })
    return in_maps


def _run_spmd(in_maps):
    nc = _build()
    res = run_bass_kernel_spmd(nc, in_maps, core_ids=list(range(N_CORES)))
    return np.stack([res.results[c]["out"] for c in range(N_CORES)])


def _assemble(outs):
    full = np.empty((B, T_FULL, C), np.float32)
    for c in range(N_CORES):
        bidx, g = divmod(c, 2)
        a = outs[c]                                  # (HPC, TC, 65, 512)
        o = a[:, :, :64, :] / a[:, :, 64:65, :]      # divide by softmax sums
        # [h, qc, d, q] -> [qc*512+q, h*64+d]
        full[bidx, :, g * 512:(g + 1) * 512] = (
            o.transpose(1, 3, 0, 2).reshape(T_FULL, 512))
    return full


def _run_spmd_main():
    """Subprocess entry: read in_maps npz from argv[1], write outs to argv[2]."""
    z = np.load(sys.argv[1])
    in_maps = []
    for c in range(N_CORES):
        m = {}
        for k in z.files:
            if not k.startswith(f"{c}_"):
                continue
            name = k[k.index("_") + 1:]
            v = z[k]
            if v.dtype.kind == "V":  # bf16 round-trips as void16
                v = v.view(ml_dtypes.bfloat16)
            m[name] = v
        in_maps.append(m)
    outs = _run_spmd(in_maps)
    np.save(sys.argv[2], outs)


def kernel(**inputs):
    in_maps = _host_prep(inputs)
    err = None
    for attempt in range(2):
        try:
            return _assemble(_run_spmd(in_maps))
        except Exception as e:          # noqa: BLE001 - device flakiness
            err = e
    # The axon device occasionally wedges for the rest of the process
    # (NRT_EXEC_UNIT_UNRECOVERABLE); a fresh process recovers it.
    import os
    import subprocess
    import tempfile
    here = os.path.dirname(os.path.abspath(__file__))
    for attempt in range(2):
        with tempfile.TemporaryDirectory() as td:
            inp, outp = os.path.join(td, "in.npz"), os.path.join(td, "out.npy")
            np.savez(inp, **{f"{c}_{k}": v for c, m in enumerate(in_maps)
                             for k, v in m.items()})
            r = subprocess.run(
                [sys.executable, "-c",
                 "import sys; sys.path.insert(0, sys.argv[3]); "
                 "import kernel; kernel._run_spmd_main()",
                 inp, outp, here],
                cwd=here, capture_output=True, text=True, timeout=1800)
            if r.returncode == 0 and os.path.exists(outp):
                return _assemble(np.load(outp))
            err = RuntimeError(
                f"subprocess kernel attempt failed: {r.stderr[-2000:]}")
    raise err
